# revision 51
# baseline (speedup 1.0000x reference)
"""Trainium2 Bass kernel for CausalSelfAttention with kron-structured bias and
column-masked causal attention.

Shapes (hardcoded): x (4,1024,512), H=8 heads, HD=64, attn_bias (8,64,64)
expanded by kron(ones(8,8)) onto the top-left 512x512 of the (1024,1024)
score matrix. Causal tril mask with every 16th column (j%16==15) zeroed.

Sharding: 8 cores = 4 batches x 2 head-groups (4 heads each). Every core runs
an identical program on its own slice:
  - Q^T,K^T (feature-major, bf16) and V (token-major, bf16; bias + ones
    column folded in via a broadcast add) for its 4 heads,
  - scores S^T = K^T.T @ Q^T per (head, 128-row key tile); for the biased
    query block the kron bias is a rank-64 second matmul (E^T, repeat(bias))
    into the same PSUM tile,
  - P^T = exp(S^T + colmask) on ScalarE (column mask via per-partition bias
    of -1e30), triangular mask on diagonal-crossing blocks via 0/1 multiply,
  - O^T_ext = V_ext.T @ P^T accumulated over key tiles (row 64 = softmax
    denominator), normalized via reciprocal + a 2-head packed broadcast
    matmul,
  - partial projection Z = y^T.T @ Wp^T slice (bf16 out). Host sums the two
    per-batch partials and adds bp.

K's bias bk is dropped entirely: it only adds a per-query constant to the
scores, which softmax cancels.

All big DMAs are issued as a handful of wide transfers split across the two
HWDGE queues (sync + scalar) so descriptor issue doesn't serialize the load
phase. Elementwise work is spread across DVE / GpSimd / ScalarE.
"""

import sys
import types

import numpy as np
import ml_dtypes

import concourse.bass as bass
import concourse.bacc as bacc
import concourse.tile as tile
from concourse import mybir
from concourse.bass_utils import run_bass_kernel_spmd


def _ensure_axon_hooks():
    """bass_utils' trace path imports antenv.axon_hooks unconditionally; some
    images lack that module. Provide it (and register the real NTFF hook when
    the axon boot shim is available) so tracing degrades gracefully."""
    try:
        import antenv.axon_hooks  # noqa: F401
        return
    except ImportError:
        pass
    m = types.ModuleType("antenv.axon_hooks")
    m._hook = None
    m.set_axon_ntff_profile_hook = lambda h: setattr(m, "_hook", h)
    m.get_axon_ntff_profile_hook = lambda: m._hook
    sys.modules["antenv.axon_hooks"] = m
    try:
        import antenv
        antenv.axon_hooks = m
    except ImportError:
        pass
    try:
        from trn_agent_boot.trn_boot import _ntff_profile_via_ctypes
        m.set_axon_ntff_profile_hook(
            _ntff_profile_via_ctypes("/opt/axon/libaxon_pjrt.so")
        )
    except Exception:
        pass


_ensure_axon_hooks()

F32 = mybir.dt.float32
F32R = mybir.dt.float32r
BF16 = mybir.dt.bfloat16
AFT = mybir.ActivationFunctionType
BFNP = ml_dtypes.bfloat16

B, T, C, H = 4, 1024, 512, 8
HD = 64
SCALE = 1.0 / 8.0
GH = 4          # heads per core
N_CORES = 8

_CACHE = {}
LAST_RESULTS = None


def _kernel_body(tc, io):
    nc = tc.nc
    from contextlib import ExitStack
    with ExitStack() as ctx:
        const = ctx.enter_context(tc.tile_pool(name="const", bufs=1))
        pmm = ctx.enter_context(tc.tile_pool(name="pmm", bufs=2, space="PSUM"))
        ps = ctx.enter_context(tc.tile_pool(name="ps", bufs=2, space="PSUM"))
        pot = ctx.enter_context(tc.tile_pool(name="pot", bufs=3, space="PSUM"))
        pbc = ctx.enter_context(tc.tile_pool(name="pbc", bufs=1, space="PSUM"))
        spt = ctx.enter_context(tc.tile_pool(name="spt", bufs=4))
        sden = ctx.enter_context(tc.tile_pool(name="sden", bufs=3))
        szout = ctx.enter_context(tc.tile_pool(name="szout", bufs=3))

        def ctile(shape, tag, dt=BF16):
            return const.tile(shape, dt, tag=tag, name=tag)

        xt = ctile([128, 4096], "xt")
        wqk = ctile([128, 2048], "wqk")
        wv = ctile([128, 1040], "wv")
        wp = ctile([128, 1024], "wp")
        et = ctile([128, 512], "et")
        bmq = ctile([128, 2048], "bmq")
        bve = ctile([128, 260], "bve", F32)
        tri = ctile([128, 128], "tri")
        cmask = ctile([128, 1], "cmask", F32)
        sel = ctile([64, 128], "sel", F32R)
        den2x = [ctile([64, 512], f"den2{i}", F32R) for i in range(2)]
        bq_t = ctile([128, 2], "bq_t", F32)
        qt = [ctile([128, 1024], f"qt{i}") for i in range(2)]
        kt = [ctile([128, 1024], f"kt{i}") for i in range(2)]
        ve = [ctile([128, 260], f"ve{i}") for i in range(8)]
        yt = [ctile([128, 1024], f"yt{i}") for i in range(2)]

        # ---- loads: few wide DMAs, split across the two HWDGE queues.
        XT, WQK, WV, WP, ET2, BMQ2 = (
            io["XT"], io["WQK"], io["WV"], io["WP"], io["ET2"], io["BMQ2"])
        BVE, TRI, CMASK, SEL, Z = (
            io["BVE"], io["TRI"], io["CMASK"], io["SEL"], io["Z"])

        # critical-path tensors first, interleaved across both HWDGE queues
        nc.sync.dma_start(out=xt[:, 0:1024], in_=XT[:, 0:1024])
        nc.scalar.dma_start(out=wqk[:, 0:1024], in_=WQK[:, 0:1024])
        nc.sync.dma_start(out=bq_t, in_=io["BQ"][:, :])
        nc.scalar.dma_start(out=xt[:, 1024:2048], in_=XT[:, 1024:2048])
        nc.sync.dma_start(out=xt[:, 2048:3072], in_=XT[:, 2048:3072])
        nc.scalar.dma_start(out=xt[:, 3072:4096], in_=XT[:, 3072:4096])
        nc.sync.dma_start(out=wqk[:, 1024:2048], in_=WQK[:, 1024:2048])
        nc.scalar.dma_start(out=wv, in_=WV[:, :])
        nc.sync.dma_start(out=et, in_=ET2[:, :])
        nc.scalar.dma_start(out=bmq, in_=BMQ2[:, :])
        nc.sync.dma_start(out=cmask, in_=CMASK[:, :])
        nc.scalar.dma_start(out=bve, in_=BVE[:, :])
        nc.sync.dma_start(out=sel, in_=SEL[:, :])
        nc.scalar.dma_start(out=tri, in_=TRI[:, :])
        nc.sync.dma_start(out=wp, in_=WP[:, :])
        nc.scalar.dma_start(out=den2x[0][:, :], in_=io["ZD"][:, :])
        nc.sync.dma_start(out=den2x[1][:, :], in_=io["ZD"][:, :])

        # ---- projection group emitters (partly woven into the attention
        # stream so the PE always has independent work during exp waits)
        def emit_qk(w, dt, ib):
            mmp = pmm.tile([128, 512], F32, tag="mm", name="mmp")
            for ct in range(4):
                nc.tensor.matmul(
                    mmp,
                    wqk[:, 1024 * w + 256 * ct + 128 * dt:
                        1024 * w + 256 * ct + 128 * dt + 128],
                    xt[:, 1024 * ct + 512 * ib:1024 * ct + 512 * ib + 512],
                    start=(ct == 0), stop=(ct == 3),
                )
            dst = (qt if w == 0 else kt)[dt][:, 512 * ib:512 * ib + 512]
            if w == 0:
                nc.vector.tensor_scalar_add(dst, mmp, bq_t[:, dt:dt + 1])
            else:
                nc.scalar.activation(dst, mmp, AFT.Copy)

        def emit_v(jt):
            vp = pmm.tile([128, 260], F32, tag="mm", name="vp")
            for ct in range(4):
                nc.tensor.matmul(
                    vp,
                    xt[:, 1024 * ct + 128 * jt:1024 * ct + 128 * jt + 128],
                    wv[:, 260 * ct:260 * ct + 260],
                    start=(ct == 0), stop=(ct == 3),
                )
            nc.vector.tensor_add(ve[jt], vp, bve)

        def emit_z(it, cast_eng=None, dma_eng=None):
            zp = pmm.tile([128, 512], F32, tag="mm", name="zp")
            for ct in range(2):
                nc.tensor.matmul(
                    zp,
                    yt[ct][:, 128 * it:128 * it + 128],
                    wp[:, 512 * ct:512 * ct + 512],
                    start=(ct == 0), stop=(ct == 1),
                )
            zs = szout.tile([128, 512], BF16, tag="z", name="zs")
            if cast_eng is None:
                nc.vector.tensor_copy(zs, zp)
            else:
                cast_eng.activation(zs, zp, AFT.Copy)
            (dma_eng or nc.sync).dma_start(
                out=Z[128 * it:128 * it + 128, :], in_=zs)

        pending_norm = None      # (blk, dt, otp_even, otp_odd, den2)

        def flush_norm():
            nonlocal pending_norm
            if pending_norm is None:
                return
            nblk, ndt, o0, o1, den2 = pending_norm
            pending_norm = None
            nq0 = nblk * 512
            bcp = pbc.tile([128, 512], F32, tag="bc", name="bcp")
            nc.tensor.matmul(bcp, sel, den2, start=True, stop=True)
            bcs = sden.tile([128, 512], F32, tag="bcs", name="bcs")
            nc.vector.reciprocal_approx_fast(out=bcs, in_=bcp)
            nc.vector.tensor_mul(
                yt[ndt][0:64, nq0:nq0 + 512], o0[0:64, :], bcs[0:64, :]
            )
            nc.vector.tensor_mul(
                yt[ndt][64:128, nq0:nq0 + 512], o1[0:64, :], bcs[64:128, :]
            )

        # attention for one (blk, head-pair), software-pipelined; `weave`
        # is a list of closures for independent PE work dripped in at two
        # points per key-tile round
        def attention_pair(blk, pair, weave, zq):
            nonlocal pending_norm
            q0 = blk * 512
            njt = 4 * (blk + 1)
            dt = pair
            hps = (2 * pair, 2 * pair + 1)
            otp2 = [pot.tile([65, 512], F32, tag="ot", name=f"otp{h}")
                    for h in hps]
            pvq = []                      # (hi, jt, c0, pt) awaiting emission

            def emit_pv():
                hi, pjt, pc0, ppt = pvq.pop(0)
                nc.tensor.matmul(
                    otp2[hi][:, pc0:],
                    ve[pjt][:, 65 * hps[hi]:65 * hps[hi] + 65],
                    ppt[:, pc0:], start=(pjt == 0), stop=(pjt == njt - 1),
                )

            for jt in range(njt):
                m = jt - 4 * blk          # >=0: diagonal-crossing tile
                c0 = 128 * m if m >= 0 else 0
                for hi in range(2):
                    hp = hps[hi]
                    off = hi * 64
                    sp = ps.tile([128, 512], F32, tag="s", name="sp")
                    has_bias = blk == 0
                    nc.tensor.matmul(
                        sp[:, c0:],
                        kt[dt][off:off + 64, 128 * jt:128 * jt + 128],
                        qt[dt][off:off + 64, q0 + c0:q0 + 512],
                        start=True, stop=not has_bias,
                    )
                    if has_bias:
                        nc.tensor.matmul(
                            sp[:, c0:],
                            et[off:off + 64, 128 * jt:128 * jt + 128],
                            bmq[off:off + 64, 512 * hp + c0:512 * hp + 512],
                            start=False, stop=True,
                        )
                    pt = spt.tile([128, 512], BF16, tag="pt", name="pt")
                    nc.scalar.activation(
                        pt[:, c0:], sp[:, c0:], AFT.Exp, bias=cmask[:, 0:1]
                    )
                    if m >= 0:
                        nc.vector.tensor_mul(
                            pt[:, c0:c0 + 128], pt[:, c0:c0 + 128], tri
                        )
                    pvq.append((hi, jt, c0, pt))
                if jt == 0:
                    flush_norm()          # previous pair's normalization
                if weave:
                    weave.pop(0)()
                if jt in (2, 5) and zq:
                    emit_z(zq.pop(0))     # weave in a blk0 Z tile
                while len(pvq) > 2:
                    emit_pv()
                if weave:
                    weave.pop(0)()
            while pvq:
                emit_pv()
            den2 = den2x[(blk * 2 + pair) % 2]
            nc.vector.tensor_copy(den2[0:1, :], otp2[0][64:65, :])
            nc.vector.tensor_copy(den2[32:33, :], otp2[1][64:65, :])
            pending_norm = (blk, dt, otp2[0], otp2[1], den2)
            return

        # head-start projections: only what blk0-pair0 needs
        emit_qk(0, 0, 0)
        emit_qk(1, 0, 0)
        for jt in range(4):
            emit_v(jt)
        # remaining projection groups dripped into blk0's attention
        weave = [
            lambda: emit_qk(0, 1, 0), lambda: emit_qk(1, 1, 0),
            lambda: emit_qk(0, 0, 1), lambda: emit_qk(1, 0, 1),
            lambda: emit_qk(0, 1, 1), lambda: emit_qk(1, 1, 1),
            lambda: emit_v(4), lambda: emit_v(5),
            lambda: emit_v(6), lambda: emit_v(7),
        ]
        attention_pair(0, 0, weave, [])
        attention_pair(0, 1, weave, [])
        zq = [0, 1, 2, 3]
        attention_pair(1, 0, weave, zq)
        attention_pair(1, 1, weave, zq)
        flush_norm()
        while zq:
            emit_z(zq.pop(0))
        for it in range(4, 8):
            emit_z(it, cast_eng=nc.scalar if it % 2 else None,
                   dma_eng=nc.scalar if it % 2 else nc.sync)


def _build():
    nc = bacc.Bacc("TRN2", target_bir_lowering=False, debug=False,
                   num_devices=N_CORES)
    io = {}

    def din(name, shape, dt=BF16):
        io[name] = nc.dram_tensor(name, shape, dt, kind="ExternalInput").ap()

    din("XT", (128, 4096))
    din("WQK", (128, 2048))
    din("WV", (128, 1040))
    din("WP", (128, 1024))
    din("ET2", (128, 512))
    din("BMQ2", (128, 2048))
    din("BVE", (128, 260), F32)
    din("TRI", (128, 128))
    din("CMASK", (128, 1), F32)
    din("SEL", (64, 128), F32R)
    din("ZD", (64, 512), F32R)
    din("BQ", (128, 2), F32)
    io["Z"] = nc.dram_tensor("Z", (T, C), BF16, kind="ExternalOutput").ap()

    with tile.TileContext(nc) as tc:
        _kernel_body(tc, io)
    nc.compile()
    return nc


def _chunk_cols(a, n):
    """(n*128, m) -> (128, n*m): stack row-chunks side by side."""
    p = a.shape[0] // n
    return np.concatenate([a[i * p:(i + 1) * p, :] for i in range(n)], axis=1)


def _host_prep(x, attn_bias, Wq, bq, Wk, bk, Wv, bv, Wp, bp):
    """Build the 8 per-core input maps."""
    f = np.float32
    ET2 = np.zeros((128, 512), f)
    for p in range(128):
        gj = p % 64
        ET2[p, gj * 8:(gj + 1) * 8] = 1.0
    TRI = (np.arange(128)[None, :] >= np.arange(128)[:, None]).astype(f)
    CMASK = np.zeros((128, 1), f)
    CMASK[15::16] = -1e30
    SEL = np.zeros((64, 128), f)
    SEL[0, 0:64] = 1.0
    SEL[32, 64:128] = 1.0

    in_maps = []
    for core in range(N_CORES):
        b, g = core // 2, core % 2
        gs = slice(256 * g, 256 * (g + 1))
        XT = _chunk_cols(np.ascontiguousarray(x[b].T, dtype=f), 4)
        wq_all = _chunk_cols(np.ascontiguousarray((Wq[gs, :] * SCALE).T, f), 4)
        wk_all = _chunk_cols(np.ascontiguousarray(Wk[gs, :].T, f), 4)
        WQK = np.concatenate([wq_all, wk_all], axis=1)
        WvT = np.zeros((C, 260), f)
        BVE = np.zeros((128, 260), f)
        for hp in range(GH):
            r = slice(256 * g + 64 * hp, 256 * g + 64 * hp + 64)
            WvT[:, 65 * hp:65 * hp + 64] = Wv[r, :].T
            BVE[:, 65 * hp:65 * hp + 64] = bv[r][None, :]
            BVE[:, 65 * hp + 64] = 1.0
        WV = _chunk_cols(WvT, 4)
        WP = _chunk_cols(np.ascontiguousarray(Wp[:, gs].T, f), 2)
        BMQ2 = np.zeros((128, 2048), f)
        for hp in range(GH):
            h = GH * g + hp
            bm = np.repeat(attn_bias[h], 8, axis=0).T.astype(f)   # (64, 512)
            BMQ2[0:64, 512 * hp:512 * hp + 512] = bm
            BMQ2[64:128, 512 * hp:512 * hp + 512] = bm
        in_maps.append({
            "XT": XT.astype(BFNP),
            "WQK": WQK.astype(BFNP),
            "WV": WV.astype(BFNP),
            "WP": WP.astype(BFNP),
            "ET2": ET2.astype(BFNP),
            "BMQ2": BMQ2.astype(BFNP),
            "BVE": BVE,
            "TRI": TRI.astype(BFNP),
            "CMASK": CMASK,
            "SEL": SEL,
            "ZD": np.zeros((64, 512), f),
            "BQ": np.ascontiguousarray(
                (bq[gs] * SCALE).reshape(2, 128).T.astype(f)),
        })
    return in_maps


def kernel(**inputs):
    global LAST_RESULTS
    if "nc" not in _CACHE:
        _CACHE["nc"] = _build()
    nc = _CACHE["nc"]

    in_maps = _host_prep(**{k: np.asarray(v) for k, v in inputs.items()})
    res = run_bass_kernel_spmd(nc, in_maps, core_ids=list(range(N_CORES)))
    LAST_RESULTS = res

    bp = np.asarray(inputs["bp"], np.float32)
    out = np.empty((B, T, C), np.float32)
    for b in range(B):
        out[b] = (np.asarray(res.results[2 * b]["Z"]).astype(np.float32)
                  + np.asarray(res.results[2 * b + 1]["Z"]).astype(np.float32)
                  + bp[None, :])
    return out
